# revision 1
# baseline (speedup 1.0000x reference)
"""DepthScaleShiftInvLoss kernel for one TRN2 chip (8 NeuronCores) — v3.

Full inputs: pred/gt f32 [32,512,512], mask bool [32,512,512].
Output: dense masked loss f32 [32,512,512] (zeros off-mask); device stores
bf16, host upcasts.

Sharding: pure data parallel — 4 samples/core across 8 cores.

v3 design (per core; SBUF layout per sample [128 x 2048], partition p holds
image rows [4p,4p+4)):
- pred/gt stream in via SWDGE (gpsimd) DMAs that cast f32->bf16 in the DMA
  datapath; all bulk DVE ops then run in 2x/4x perf modes. mask u8 + out bf16
  ride the SP HWDGE queue, keeping the two DMA paths independent.
- ACT does 3 passes/sample: mask cast (accum -> count) and the two
  |x - shift| accum passes (1x rate, dtype-independent).
- DVE bulk/sample: pm=p*mf, gm=g*mf (TT 2x, feeding PE masked sums),
  u=r*g+q (TS dual 4x), v=p-u (TT 2x), z=|v|*a (TS abs_max+mult 4x),
  out=z*mf (TT 2x). scalar_tensor_tensor is avoided (no DVE perf modes).
- PE: per-partition masked sums via 16 accumulating matmuls (data stationary,
  ones moving), plus tiny ones[128,128] matmuls folding partials across
  partitions at the two stats barriers.
- stage C algebra: out = |(p-sp)/s_p - (g-sg)/s_g|*m = |a*(p - r*g - q)|*m
  with r=s_p/s_g, q=sp-r*sg, a=1/s_p; off-mask lanes hold garbage until the
  final mask multiply zeroes them.
"""

import numpy as np

import concourse.bass as bass
import concourse.bacc as bacc
import concourse.tile as tile
from concourse import mybir
from concourse.bass_utils import run_bass_kernel_spmd

B, H, W = 32, 512, 512
N_CORES = 8
B_LOC = B // N_CORES          # samples per core
P = 128                       # SBUF partitions
FD = (H // P) * W             # free elements per sample per partition
N_ELEM = float(H * W)         # elements per sample
EPS = 1e-6

f32 = mybir.dt.float32
bf16 = mybir.dt.bfloat16
u8 = mybir.dt.uint8

ALU = mybir.AluOpType
ACTF = mybir.ActivationFunctionType


class _PerSample:
    __slots__ = ("mf", "pin", "gin", "pm", "gm", "st1", "p2",
                 "cnt", "invc", "spsg", "corr", "a_p", "r_t", "q_t")


def build_body(nc):
    pred = nc.dram_tensor("pred", [B_LOC, H, W], f32, kind="ExternalInput").ap()
    gt = nc.dram_tensor("gt", [B_LOC, H, W], f32, kind="ExternalInput").ap()
    mask = nc.dram_tensor("mask", [B_LOC, H, W], u8, kind="ExternalInput").ap()
    out = nc.dram_tensor("out", [B_LOC, H, W], bf16, kind="ExternalOutput").ap()

    pr = pred.rearrange("a (p r) w -> p a (r w)", p=P)
    gr = gt.rearrange("a (p r) w -> p a (r w)", p=P)
    mr = mask.rearrange("a (p r) w -> p a (r w)", p=P)
    outr = out.rearrange("a (p r) w -> p a (r w)", p=P)

    with tile.TileContext(nc) as tc:
        with (
            tc.tile_pool(name="io", bufs=4) as io,
            tc.tile_pool(name="keep", bufs=B_LOC) as keep,
            tc.tile_pool(name="tmp", bufs=2) as tmp,
            tc.tile_pool(name="small", bufs=B_LOC) as small,
            tc.tile_pool(name="ps", bufs=2, space="PSUM") as ps,
            tc.tile_pool(name="const", bufs=1) as const,
        ):
            ones = const.tile([P, P], f32)
            nc.vector.memset(ones, 1.0)
            ones_b = const.tile([P, 1], bf16)
            nc.vector.memset(ones_b, 1.0)

            S = [_PerSample() for _ in range(B_LOC)]
            eng = nc.vector

            def pe_sum(big, psum_acc):
                # psum_acc[m] = sum_p,k big[p, 128k+m]; folded across
                # partitions later by a ones[128,128] matmul
                for k in range(0, FD, P):
                    nc.tensor.matmul(psum_acc, big[:, k:k + P], ones_b,
                                     start=(k == 0), stop=(k == FD - P))

            def stage_mask(s):
                st = S[s]
                m_in = io.tile([P, FD], u8, tag="m_in", bufs=B_LOC, name=f"m_in{s}")
                nc.sync.dma_start(out=m_in, in_=mr[:, s, :])
                # st1 cols: [count, sum(pm), sum(gm)] per-partition partials
                st.st1 = small.tile([P, 3], f32, tag="st1", name=f"st1_{s}")
                st.mf = keep.tile([P, FD], bf16, tag="mf", name=f"mf{s}")
                nc.scalar.activation(out=st.mf, in_=m_in, func=ACTF.Copy,
                                     accum_out=st.st1[:, 0:1])

            def stage_a(s):
                st = S[s]
                st.pin = io.tile([P, FD], bf16, tag="p_in", name=f"p_in{s}")
                nc.gpsimd.dma_start(out=st.pin, in_=pr[:, s, :])
                st.gin = io.tile([P, FD], bf16, tag="g_in", name=f"g_in{s}")
                nc.gpsimd.dma_start(out=st.gin, in_=gr[:, s, :])

                st.pm = keep.tile([P, FD], bf16, tag="pm", bufs=3, name=f"pm{s}")
                eng.tensor_tensor(st.pm, st.pin, st.mf, ALU.mult)
                st.gm = keep.tile([P, FD], bf16, tag="gm", bufs=3, name=f"gm{s}")
                eng.tensor_tensor(st.gm, st.gin, st.mf, ALU.mult)
                with tc.high_priority():
                    psum_s = ps.tile([P, 2], f32, tag="psum_s", name=f"pss{s}")
                    pe_sum(st.pm, psum_s[:, 0:1])
                    pe_sum(st.gm, psum_s[:, 1:2])
                    eng.tensor_copy(st.st1[:, 1:3], psum_s)

            def barrier1(s):
                st = S[s]
                with tc.high_priority():
                    psum1 = ps.tile([P, 3], f32, tag="psum1", name=f"ps1_{s}")
                    nc.tensor.matmul(psum1, ones, st.st1, start=True, stop=True)
                    st.cnt = small.tile([P, 1], f32, tag="cnt", name=f"cnt{s}")
                    eng.tensor_scalar(st.cnt, psum1[:, 0:1], 1.0, None, ALU.max)
                    st.invc = small.tile([P, 1], f32, tag="invc", name=f"invc{s}")
                    nc.vector.reciprocal(st.invc, st.cnt)
                    st.spsg = small.tile([P, 2], f32, tag="spsg", name=f"spsg{s}")
                    eng.tensor_scalar(st.spsg, psum1[:, 1:3], st.invc, None,
                                      ALU.mult)

            def stage_b(s):
                st = S[s]
                st.p2 = small.tile([P, 2], f32, tag="p2", name=f"p2_{s}")
                scr = tmp.tile([P, FD], bf16, tag="scr", name=f"scr{s}")
                nc.scalar.activation(
                    out=scr, in_=st.pm, func=ACTF.Abs,
                    bias=st.spsg[:, 0:1], scale=-1.0, accum_out=st.p2[:, 0:1])
                scr2 = tmp.tile([P, FD], bf16, tag="scr", name=f"scr2_{s}")
                nc.scalar.activation(
                    out=scr2, in_=st.gm, func=ACTF.Abs,
                    bias=st.spsg[:, 1:2], scale=-1.0, accum_out=st.p2[:, 1:2])
                # off-mask elements contribute |shift| each; correction
                # (N-cnt)*|shift| computed off the barrier-2 critical path
                with tc.high_priority():
                    asps = small.tile([P, 2], f32, tag="asps", name=f"asps{s}")
                    nc.scalar.activation(out=asps, in_=st.spsg, func=ACTF.Abs)
                    offc = small.tile([P, 1], f32, tag="offc", name=f"offc{s}")
                    eng.tensor_scalar(offc, st.cnt, -1.0, N_ELEM,
                                      ALU.mult, ALU.add)
                    st.corr = small.tile([P, 2], f32, tag="corr", name=f"corr{s}")
                    eng.tensor_scalar(st.corr, asps, offc, None, ALU.mult)

            def barrier2(s):
                st = S[s]
                with tc.high_priority():
                    psum2 = ps.tile([P, 2], f32, tag="psum2", name=f"ps2_{s}")
                    nc.tensor.matmul(psum2, ones, st.p2, start=True, stop=True)
                    num = small.tile([P, 2], f32, tag="num", name=f"num{s}")
                    eng.tensor_tensor(num, psum2, st.corr, ALU.subtract)
                    scpg = small.tile([P, 2], f32, tag="scpg", name=f"scpg{s}")
                    eng.tensor_scalar(scpg, num, st.invc, EPS, ALU.mult, ALU.max)
                    ipg = small.tile([P, 2], f32, tag="ipg", name=f"ipg{s}")
                    nc.vector.reciprocal(ipg, scpg)
                    st.a_p = ipg[:, 0:1]
                    st.r_t = small.tile([P, 1], f32, tag="r_t", name=f"rt{s}")
                    eng.tensor_tensor(st.r_t, scpg[:, 0:1], ipg[:, 1:2], ALU.mult)
                    rsg = small.tile([P, 1], f32, tag="rsg", name=f"rsg{s}")
                    eng.tensor_tensor(rsg, st.r_t, st.spsg[:, 1:2], ALU.mult)
                    st.q_t = small.tile([P, 1], f32, tag="q_t", name=f"qt{s}")
                    eng.tensor_tensor(st.q_t, st.spsg[:, 0:1], rsg, ALU.subtract)

            def stage_c(s, splits=1):
                st = S[s]
                # splits>1 pipelines column chunks so the tail sample's
                # compute -> out-DMA chain is shorter
                cw = FD // splits
                for h in range(splits):
                    sl = slice(h * cw, (h + 1) * cw)
                    u = tmp.tile([P, cw], bf16, tag="u", name=f"u{s}_{h}")
                    eng.tensor_scalar(u, st.gin[:, sl], st.r_t, st.q_t,
                                      ALU.mult, ALU.add)
                    v = tmp.tile([P, cw], bf16, tag="v", name=f"v{s}_{h}")
                    eng.tensor_tensor(v, st.pin[:, sl], u, ALU.subtract)
                    # signed a*(p-r*g-q); |.| applied on host (mask >= 0)
                    z = tmp.tile([P, cw], bf16, tag="z", name=f"z{s}_{h}")
                    eng.tensor_scalar(z, v, st.a_p, None, ALU.mult)
                    outt = tmp.tile([P, cw], bf16, tag="outt", name=f"outt{s}_{h}")
                    eng.tensor_tensor(outt, z, st.mf[:, sl], ALU.mult)
                    nc.sync.dma_start(out=outr[:, s, sl], in_=outt)

            # Emission order == scheduling priority. All mask casts head the
            # ACT queue (their inputs are dep-free); samples staggered so one
            # sample's stats barriers hide behind other samples' bulk work.
            stage_mask(0)
            stage_mask(1)
            stage_mask(2)
            stage_mask(3)
            stage_a(0)
            stage_a(1)
            barrier1(0)
            stage_b(0)
            stage_a(2)
            barrier1(1)
            stage_b(1)
            barrier2(0)
            stage_a(3)
            barrier1(2)
            stage_b(2)
            barrier2(1)
            stage_c(0)
            barrier1(3)
            stage_b(3)
            barrier2(2)
            stage_c(1)
            barrier2(3)
            stage_c(2, splits=2)
            stage_c(3, splits=2)
    return nc


_CACHED = None


def _get_nc():
    global _CACHED
    if _CACHED is None:
        nc = bacc.Bacc("TRN2", target_bir_lowering=False, debug=False)
        build_body(nc)
        nc.compile()
        _CACHED = nc
    return _CACHED


def kernel(pred: np.ndarray, gt: np.ndarray, mask: np.ndarray) -> np.ndarray:
    pred = np.ascontiguousarray(np.asarray(pred), dtype=np.float32)
    gt = np.ascontiguousarray(np.asarray(gt), dtype=np.float32)
    mask = np.asarray(mask)
    mask_u8 = np.ascontiguousarray(
        mask.view(np.uint8) if mask.dtype == np.bool_ else mask.astype(np.uint8)
    )

    nc = _get_nc()
    in_maps = []
    for c in range(N_CORES):
        lo, hi = c * B_LOC, (c + 1) * B_LOC
        in_maps.append(
            {"pred": pred[lo:hi], "gt": gt[lo:hi], "mask": mask_u8[lo:hi]}
        )
    res = run_bass_kernel_spmd(nc, in_maps, core_ids=list(range(N_CORES)))
    full = np.concatenate([res.results[c]["out"] for c in range(N_CORES)], axis=0)
    return np.abs(full.astype(np.float32))

